# revision 1
# baseline (speedup 1.0000x reference)
"""Multi-head attention block (dense transformer) on 8 trn2 NeuronCores.

Sharding: batch (4) x head-group (2 groups of 8 heads) = 8 cores. Each core
computes, for its batch b and its 8 heads:
    qkv slice -> per-head softmax(q k^T / sqrt(D)) v -> partial out proj.
Host sums the two head-group partials per batch and adds the output bias.

Device dataflow is fully "transposed": the projection produces qT/kT with
head-dim on partitions (what the S^T matmul wants) and V in natural layout
with a fused ones-column, so P @ V also yields the softmax denominators.
exp() runs on the scalar engine straight out of PSUM in [128, 1024] windows.
No max-subtraction: logits are ~N(0, 0.25) by construction, exp is safe.
"""

import numpy as np
import ml_dtypes
import jax
import jax.core
from jax.experimental.shard_map import shard_map
from jax.sharding import Mesh, PartitionSpec

import concourse.bass as bass
import concourse.mybir as mybir
import concourse.tile as tile
import concourse.bass2jax as bass2jax
from concourse.vector_clock import ScopedClock

# ---------------------------------------------------------------------------
# Workaround for the pinned walrus compiler: it rejects instructions carrying
# more than one sync wait. Split extra waits onto NOPs inserted immediately
# before the instruction in the same engine stream (identical semantics: the
# engine blocks on each wait in turn).
# ---------------------------------------------------------------------------
_MAX_WAITS = 1
_patched = False


def _split_waits(ordered):
    for bb_name, insts in ordered.items():
        out = []
        for inst in insts:
            si = inst.sync_info
            waits = list(si.on_wait) if si and si.on_wait else []
            if len(waits) > _MAX_WAITS:
                rest, keep = waits[:-_MAX_WAITS], waits[-_MAX_WAITS:]
                for k in range(0, len(rest), _MAX_WAITS):
                    out.append(mybir.InstNoOp(
                        name=f"{inst.name}-wsplit{k}",
                        sync_info=mybir.SyncInfo(
                            on_wait=rest[k:k + _MAX_WAITS], on_update=[]),
                        bass_nofuse=True,
                        engine=inst.engine,
                    ))
                inst.sync_info = mybir.SyncInfo(
                    on_wait=keep, on_update=list(si.on_update or []))
            out.append(inst)
        ordered[bb_name] = out
    return ordered


def _install_patches():
    global _patched
    if _patched:
        return
    _patched = True

    orig_lower = tile.TileContext._lower_ordered_insts

    def lower_with_split(self, ordered):
        return orig_lower(self, _split_waits(ordered))

    tile.TileContext._lower_ordered_insts = lower_with_split

    def drain_and_barrier(self, tick_clock, wait_clock):
        nc = self.nc
        drain_inst = nc.sync.drain()
        wait_clock.add_sem_waits(
            drain_inst.ins, ScopedClock({None: tick_clock.global_clock}))
        si = drain_inst.ins.sync_info
        waits = list(si.on_wait) if si and si.on_wait else []
        upds = list(si.on_update) if si and si.on_update else []
        if len(waits) > _MAX_WAITS:
            drain_inst.ins.sync_info = mybir.SyncInfo(
                on_wait=waits[:_MAX_WAITS], on_update=upds)
            for i in range(_MAX_WAITS, len(waits), _MAX_WAITS):
                nop = nc.sync.nop()
                nop.ins.sync_info = mybir.SyncInfo(
                    on_wait=waits[i:i + _MAX_WAITS], on_update=[])
        nc.all_engine_barrier()
        assert self.sems is not None
        popped = nc._tile_sem_poison_stack.pop()
        assert popped is self._sem_poison
        nc.clear_and_free_semaphores(list(self.sems.allocated().values()))
        nc.all_engine_barrier()

    tile.TileContext._drain_and_barrier = drain_and_barrier


# ---------------------------------------------------------------------------
# Problem constants (hardcoded per the task contract).
# ---------------------------------------------------------------------------
B, N, D, H, HD = 4, 2048, 1024, 16, 64
NCORES = 8
HPC = 8                 # heads per core
NPAIRS = HPC // 2       # head pairs per core
KD = D // 128           # 8 contraction tiles for the projections
NJ = N // 128           # 16 key tiles
NIC = N // 512          # 4 query chunks of 512
NT = N // 128           # 16 output row tiles
SCALE = float(D) ** -0.5

BF16 = mybir.dt.bfloat16
F32 = mybir.dt.float32
FT = mybir.ActivationFunctionType


def build_nc(loop_n: int = 1, exp_split: bool = False, no_exp: bool = False,
             st_k128: bool = False, spread_proj: bool = True) -> bass.Bass:
    """loop_n > 1 wraps the whole body in a hardware loop (benchmark builds
    only) so per-iteration device time can be extracted from wall clock."""
    _install_patches()
    nc = bass.Bass()

    xt = nc.dram_tensor("xt", [D, N], BF16, kind="ExternalInput")
    wqk = nc.dram_tensor("wqk", [D, 1024], BF16, kind="ExternalInput")
    wv = nc.dram_tensor("wv", [D, 512], BF16, kind="ExternalInput")
    wo = nc.dram_tensor("wo", [512, D], BF16, kind="ExternalInput")
    out = nc.dram_tensor("out", [N, D], F32, kind="ExternalOutput")
    # per-(head, i-chunk) softmax denominator rows, bounced through DRAM to
    # broadcast across partitions
    rsums = nc.dram_tensor("rsums", [HPC * NIC, 512], F32, kind="Internal")

    import contextlib

    with tile.TileContext(nc) as tc:
        loop_ctx = (tc.For_i(0, loop_n, 1,
                             hint_engines=(mybir.EngineType.PE,
                                           mybir.EngineType.Activation,
                                           mybir.EngineType.DVE,
                                           mybir.EngineType.SP))
                    if loop_n > 1 else contextlib.nullcontext())
        with (
            loop_ctx,
            tc.tile_pool(name="persist", bufs=1) as pers,
            tc.tile_pool(name="expp", bufs=2, space="PSUM") as expp,
            tc.tile_pool(name="pvp", bufs=2, space="PSUM") as pvp,
            tc.tile_pool(name="mmp", bufs=2, space="PSUM") as mmp,
            tc.tile_pool(name="utp", bufs=22) as utp,
            tc.tile_pool(name="pvstage", bufs=6) as pvstage,
            tc.tile_pool(name="rp", bufs=4) as rp,
            tc.tile_pool(name="fstage", bufs=4) as fstage,
        ):
            # ---- persistent SBUF tensors -----------------------------------
            xt_sb = [pers.tile([128, N], BF16, tag=f"xt{i}", name=f"xt{i}") for i in range(KD)]
            wqk_sb = [pers.tile([128, 1024], BF16, tag=f"wqk{i}", name=f"wqk{i}") for i in range(KD)]
            wv_sb = [pers.tile([128, 512], BF16, tag=f"wv{i}", name=f"wv{i}") for i in range(KD)]
            wo_sb = [pers.tile([128, D], BF16, tag=f"wo{i}", name=f"wo{i}") for i in range(4)]
            qkT_sb = [pers.tile([128, N], BF16, tag=f"qk{i}", name=f"qk{i}") for i in range(8)]
            vp_sb = [pers.tile([128, HPC, HD + 1], BF16, tag=f"vp{i}", name=f"vp{i}")
                     for i in range(NJ)]
            ot_sb = [pers.tile([128, N], BF16, tag=f"ot{i}", name=f"ot{i}") for i in range(NPAIRS)]

            for i in range(KD):
                nc.sync.dma_start(out=wqk_sb[i], in_=wqk[i * 128:(i + 1) * 128, :])
            for i in range(KD):
                nc.sync.dma_start(out=xt_sb[i], in_=xt[i * 128:(i + 1) * 128, :])
            for i in range(KD):
                nc.sync.dma_start(out=wv_sb[i], in_=wv[i * 128:(i + 1) * 128, :])
            for i in range(4):
                nc.sync.dma_start(out=wo_sb[i], in_=wo[i * 128:(i + 1) * 128, :])
            for j in range(NJ):
                nc.vector.memset(vp_sb[j][:, :, HD:HD + 1], 1.0)

            # ---- stage A helpers -------------------------------------------
            def project_v():
                # V natural layout; emitted after the first exp stream is
                # underway so ACT ramps up as early as possible.
                for j in range(NJ):
                    ps = mmp.tile([128, 512], F32, tag="mm512", name=f"psv{j}")
                    for kd in range(KD):
                        nc.tensor.matmul(
                            ps,
                            xt_sb[kd][:, j * 128:(j + 1) * 128],
                            wv_sb[kd],
                            start=(kd == 0), stop=(kd == KD - 1))
                    nc.vector.tensor_copy(
                        vp_sb[j][:, :, 0:HD],
                        ps.rearrange("p (h d) -> p h d", h=HPC))
            def project_group(ct, ic):
                ps = mmp.tile([128, 512], F32, tag="mm512", name=f"psq{ct}{ic}")
                for kd in range(KD):
                    nc.tensor.matmul(
                        ps,
                        wqk_sb[kd][:, ct * 128:(ct + 1) * 128],
                        xt_sb[kd][:, ic * 512:(ic + 1) * 512],
                        start=(kd == 0), stop=(kd == KD - 1))
                nc.vector.tensor_copy(
                    qkT_sb[ct][:, ic * 512:(ic + 1) * 512], ps)

            # projection work units for pair hp: kT first (S^T j-loop needs
            # all of kT), then qT
            def pair_proj_units(hp):
                return [(4 + hp, ic) for ic in range(NIC)] +                        [(hp, ic) for ic in range(NIC)]

            # ---- per-pair pipeline.  Pair hp+1's projection groups are
            # ---- spread across pair hp's exp phases so PE has filler work
            # ---- while ACT streams. ----------------------------------------
            for ct, icg in pair_proj_units(0):
                project_group(ct, icg)
            for hp in range(NPAIRS):
                if not spread_proj and hp + 1 < NPAIRS:
                    for ct, icg in pair_proj_units(hp + 1):
                        project_group(ct, icg)
                kT = qkT_sb[4 + hp]
                qT = qkT_sb[hp]
                nxt = pair_proj_units(hp + 1) if hp + 1 < NPAIRS else []
                for ic in range(NIC):
                    qsA = qT[0:64, ic * 512:(ic + 1) * 512]
                    qsB = qT[64:128, ic * 512:(ic + 1) * 512]
                    uts = []
                    for j in range(NJ):
                        ps = expp.tile([128, 1024], F32, tag="exps", name=f"se{hp}{ic}{j}")
                        if st_k128:
                            # timing-only: one K=128 matmul instead of the
                            # K=64 pair (wrong numerics, half the mm count)
                            nc.tensor.matmul(
                                ps[:, 0:512],
                                kT[:, j * 128:(j + 1) * 128],
                                qT[:, ic * 512:(ic + 1) * 512],
                                start=True, stop=True)
                            nc.tensor.matmul(
                                ps[:, 512:1024],
                                kT[:, j * 128:(j + 1) * 128],
                                qT[:, ic * 512:(ic + 1) * 512],
                                start=True, stop=True)
                        else:
                            nc.tensor.matmul(
                                ps[:, 0:512],
                                kT[0:64, j * 128:(j + 1) * 128], qsA,
                                start=True, stop=True)
                            nc.tensor.matmul(
                                ps[:, 512:1024],
                                kT[64:128, j * 128:(j + 1) * 128], qsB,
                                start=True, stop=True, tile_position=(64, 0))
                        ut = utp.tile([128, 1024], BF16, tag="ut", name=f"ut{hp}{ic}{j}")
                        if no_exp:
                            # timing-only variant: unload ACT entirely
                            nc.vector.tensor_copy(ut, ps)
                        elif exp_split:
                            nc.scalar.activation(out=ut[:, 0:512],
                                                 in_=ps[:, 0:512], func=FT.Exp)
                            nc.scalar.activation(out=ut[:, 512:1024],
                                                 in_=ps[:, 512:1024], func=FT.Exp)
                        else:
                            nc.scalar.activation(out=ut, in_=ps, func=FT.Exp)
                        uts.append(ut)
                    if hp == 0 and ic == 0:
                        project_v()
                    elif ic >= 1 and spread_proj:
                        # 3/3/2 projection groups of the next pair
                        share = nxt[3 * (ic - 1):3 * ic] if ic < 3 else nxt[6:]
                        for ct, icg in share:
                            project_group(ct, icg)
                    for hh in range(2):
                        hloc = 2 * hp + hh
                        c0 = 512 * hh
                        pvt = pvp.tile([HD + 1, 512], F32, tag="pv", name=f"pv{hloc}{ic}")
                        for j in range(NJ):
                            nc.tensor.matmul(
                                pvt,
                                vp_sb[j][:, hloc, :],
                                uts[j][:, c0:c0 + 512],
                                start=(j == 0), stop=(j == NJ - 1))
                        stg = pvstage.tile([HD + 1, 512], F32, tag="pvs", name=f"st{hloc}{ic}")
                        nc.vector.tensor_copy(stg, pvt)
                        hic = hloc * NIC + ic
                        nc.sync.dma_start(out=rsums[hic:hic + 1, :],
                                          in_=stg[HD:HD + 1, :])
                        rt = rp.tile([HD, 512], F32, tag="rt", name=f"rt{hloc}{ic}")
                        srcap = rsums[hic:hic + 1, :]
                        nc.sync.dma_start(out=rt, in_=bass.AP(
                            tensor=srcap.tensor, offset=srcap.offset,
                            ap=[[0, HD]] + list(srcap.ap[1:])))
                        nc.vector.reciprocal(rt, rt)
                        nc.vector.tensor_mul(
                            ot_sb[hp][64 * hh:64 * hh + 64,
                                      ic * 512:(ic + 1) * 512],
                            stg[0:HD, :], rt)

                    if hp == NPAIRS - 1:
                        # all pairs have this i-chunk done: project it out
                        for it in range(4 * ic, 4 * ic + 4):
                            for oc in range(2):
                                ps = mmp.tile([128, 512], F32, tag="mm512",
                                              name=f"psf{it}{oc}")
                                for kt in range(4):
                                    nc.tensor.matmul(
                                        ps,
                                        ot_sb[kt][:, it * 128:(it + 1) * 128],
                                        wo_sb[kt][:, oc * 512:(oc + 1) * 512],
                                        start=(kt == 0), stop=(kt == 3))
                                fs = fstage.tile([128, 512], F32, tag="fs",
                                                 name=f"fs{it}{oc}")
                                nc.vector.tensor_copy(fs, ps)
                                nc.sync.dma_start(
                                    out=out[it * 128:(it + 1) * 128,
                                            oc * 512:(oc + 1) * 512],
                                    in_=fs)

    return nc


# ---------------------------------------------------------------------------
# Cached SPMD runner (replicates bass2jax.run_bass_via_pjrt's multi-core path
# but jits once so repeated calls don't recompile).
# ---------------------------------------------------------------------------
_RUNNER = None


def _build_runner():
    nc = build_nc()
    bass2jax.install_neuronx_cc_hook()

    partition_name = (nc.partition_id_tensor.name
                      if nc.partition_id_tensor else None)
    in_names, out_names, out_avals, zero_shapes = [], [], [], []
    for alloc in nc.m.functions[0].allocations:
        if not isinstance(alloc, mybir.MemoryLocationSet):
            continue
        name = alloc.memorylocations[0].name
        if alloc.kind == "ExternalInput":
            if name != partition_name:
                in_names.append(name)
        elif alloc.kind == "ExternalOutput":
            shape = tuple(alloc.tensor_shape)
            dtype = mybir.dt.np(alloc.dtype)
            out_names.append(name)
            out_avals.append(jax.core.ShapedArray(shape, dtype))
            zero_shapes.append((shape, dtype))
    n_params = len(in_names)
    n_outs = len(out_avals)
    all_in_names = list(in_names) + list(out_names)
    if partition_name is not None:
        all_in_names.append(partition_name)

    def _body(*args):
        operands = list(args)
        if partition_name is not None:
            operands.append(bass2jax.partition_id_tensor())
        outs = bass2jax._bass_exec_p.bind(
            *operands,
            out_avals=tuple(out_avals),
            in_names=tuple(all_in_names),
            out_names=tuple(out_names),
            lowering_input_output_aliases=(),
            sim_require_finite=True,
            sim_require_nnan=True,
            nc=nc,
        )
        return tuple(outs)

    devices = jax.devices()[:NCORES]
    mesh = Mesh(np.asarray(devices), ("core",))
    in_specs = (PartitionSpec("core"),) * (n_params + n_outs)
    out_specs = (PartitionSpec("core"),) * n_outs
    donate = tuple(range(n_params, n_params + n_outs))
    sharded = jax.jit(
        shard_map(_body, mesh=mesh, in_specs=in_specs, out_specs=out_specs,
                  check_rep=False),
        donate_argnums=donate, keep_unused=True)

    def run(in_maps):
        concat_in = [
            np.concatenate([np.asarray(in_maps[c][nm]) for c in range(NCORES)],
                           axis=0)
            for nm in in_names
        ]
        concat_zeros = [np.zeros((NCORES * s[0], *s[1:]), dt)
                        for (s, dt) in zero_shapes]
        out_arrs = sharded(*concat_in, *concat_zeros)
        out_arrs = [np.asarray(a) for a in out_arrs]
        return [
            {nm: out_arrs[i].reshape(NCORES, *out_avals[i].shape)[c]
             for i, nm in enumerate(out_names)}
            for c in range(NCORES)
        ]

    return run


def _prep_inputs(x, w_qkv, w_out):
    """Host-side shard prep: per-core xt / wqk / wv / wo in bf16."""
    x = np.asarray(x, dtype=np.float32)
    w_qkv = np.asarray(w_qkv, dtype=np.float32)
    w_out = np.asarray(w_out, dtype=np.float32)

    w3 = w_qkv.reshape(D, 3, H, HD)
    wq, wk, wv_ = w3[:, 0], w3[:, 1], w3[:, 2]
    wo_h = w_out.reshape(H, HD, D)

    in_maps = []
    for c in range(NCORES):
        b, g = divmod(c, 2)
        hs = slice(8 * g, 8 * g + 8)
        xt = np.ascontiguousarray(x[b].T).astype(ml_dtypes.bfloat16)
        wqk = np.concatenate([
            (wq[:, hs] * SCALE).reshape(D, 512),
            wk[:, hs].reshape(D, 512),
        ], axis=1).astype(ml_dtypes.bfloat16)
        wv = wv_[:, hs].reshape(D, 512).astype(ml_dtypes.bfloat16)
        wo = wo_h[hs].reshape(512, D).astype(ml_dtypes.bfloat16)
        in_maps.append({"xt": xt, "wqk": wqk, "wv": wv, "wo": wo})
    return in_maps


def get_runner():
    global _RUNNER
    if _RUNNER is None:
        _RUNNER = _build_runner()
    return _RUNNER


def kernel(x, w_qkv, w_out, b_out):
    b_out = np.asarray(b_out, dtype=np.float32)
    in_maps = _prep_inputs(x, w_qkv, w_out)
    results = get_runner()(in_maps)
    out = np.empty((B, N, D), dtype=np.float32)
    for b in range(B):
        out[b] = results[2 * b]["out"] + results[2 * b + 1]["out"] + b_out
    return out

